# revision 4
# baseline (speedup 1.0000x reference)
"""Causal self-attention (B=4, T=2048, C=1024, H=16) on 8 Trainium2 NeuronCores.

Sharding (per the hint): data-parallel over batch (4) x tensor-parallel over
head halves (2) = 8 cores. Core c handles batch b = c//2 and heads
[8*(c%2), 8*(c%2)+8). Each core computes:
  - qkv projection for its 8 heads from x[b]^T (transposed on host)
  - causal attention in a fully transposed layout:
      scores^T[key, q] = k_chunk @ q^T   (no transposes needed anywhere)
      probs^T = exp(scale * scores^T) * causal_mask
      out^T[d, q]  accumulated as v_aug^T @ probs^T, where v_aug has a ones
      column so row 64 of the accumulator is the softmax denominator
  - partial out-projection with its 512-row slice of w_out
Host sums the two partial outputs per batch element (the tensor-parallel
all-reduce done on host, since the output must be gathered anyway).

All matmul operands use dtype float32r (fp32 bits, PE streams them at 1
row/cycle instead of 4; inputs are rounded to ~19 mantissa bits by the
producing engine, accumulation stays fp32 in PSUM).
"""
import sys

if "/opt/trn_rl_repo" not in sys.path:
    sys.path.insert(0, "/opt/trn_rl_repo")

import numpy as np

T = 2048
C = 1024
HLOC = 8          # heads per core
DK = 64
HD = HLOC * DK    # 512 local head dims
KC = C // 128     # 8 contraction chunks for the qkv projection
NMT = HD // 128   # 4 tiles of q^T / k^T rows
NVT = T // 128    # 16 v tiles
NQT = T // 512    # 4 q tiles of 512
SCALE = DK ** -0.5

_CACHE = {}


def _build_nc():
    import concourse.mybir as mybir
    import concourse.tile as tile
    from concourse import bacc
    from concourse.masks import make_upper_triangular

    F32 = mybir.dt.float32
    F32R = mybir.dt.float32r
    AF = mybir.ActivationFunctionType

    nc = bacc.Bacc("TRN2", target_bir_lowering=False, debug=False, num_devices=8)
    xT = nc.dram_tensor("xT", [C, T], F32R, kind="ExternalInput")
    wq = nc.dram_tensor("wq", [C, HD], F32R, kind="ExternalInput")
    wk = nc.dram_tensor("wk", [C, HD], F32R, kind="ExternalInput")
    wv = nc.dram_tensor("wv", [C, HD], F32R, kind="ExternalInput")
    wo = nc.dram_tensor("wo", [HD, C], F32R, kind="ExternalInput")
    y = nc.dram_tensor("y", [T, C], F32, kind="ExternalOutput")

    with tile.TileContext(nc) as tc:
        with tc.tile_pool(name="const", bufs=1) as const, \
             tc.tile_pool(name="qkv", bufs=1) as qkv, \
             tc.tile_pool(name="ps", bufs=2, space="PSUM") as psp:
            # constants
            mask_f = const.tile([128, 128], F32)            # 1 where key <= q
            make_upper_triangular(nc, mask_f, val=1.0, diag=True)
            ones_f = const.tile([128, 64], F32)
            nc.vector.memset(ones_f, 1.0)
            ones_r = const.tile([1, 64], F32R)
            nc.vector.tensor_copy(ones_r, ones_f[0:1, :])
            onecol_f = const.tile([128, 1], F32)
            nc.vector.memset(onecol_f, 1.0)

            # outputs of the qkv projection, live through phases 1-3
            qT_sb = qkv.tile([128, NMT, T], F32R)           # q^T: [head_dim, t]
            kT_sb = qkv.tile([128, NMT, T], F32R)
            v_sb = qkv.tile([128, NVT, HLOC * 65], F32R)    # v_aug: ones col per head

            # ---------------- phase 1: qkv projection ----------------
            with tc.tile_pool(name="xtw", bufs=1) as xtw, \
                 tc.tile_pool(name="wpool", bufs=1) as wpool:
                xT_sb = xtw.tile([128, KC, T], F32R)
                for kc in range(KC):
                    nc.sync.dma_start(out=xT_sb[:, kc, :],
                                      in_=xT.ap()[kc * 128:(kc + 1) * 128, :])

                ps_tags = ["sc", "aux", "oa"]

                def ph1_psum(i):
                    return psp.tile([128, 512], F32, tag=ps_tags[i % 3],
                                    name=f"ps1_{i}")

                pscnt = 0
                # q^T and k^T: out = w_slice.T @ x^T (moving = x^T)
                for w_dram, outT in ((wq, qT_sb), (wk, kT_sb)):
                    w_sb = wpool.tile([128, KC, HD], F32R, tag="w")
                    w_re = w_dram.rearrange("(kc p) n -> p kc n", p=128)
                    for kc in range(KC):
                        nc.sync.dma_start(out=w_sb[:, kc, :], in_=w_re[:, kc, :])
                    for mt in range(NMT):
                        for nt in range(NQT):
                            ps = ph1_psum(pscnt); pscnt += 1
                            for kc in range(KC):
                                nc.tensor.matmul(
                                    ps, w_sb[:, kc, mt * 128:(mt + 1) * 128],
                                    xT_sb[:, kc, nt * 512:(nt + 1) * 512],
                                    start=(kc == 0), stop=(kc == KC - 1))
                            nc.scalar.copy(outT[:, mt, nt * 512:(nt + 1) * 512], ps)
                # v: out = x @ wv (stationary = x^T chunks)
                w_sb = wpool.tile([128, KC, HD], F32R, tag="w")
                w_re = wv.rearrange("(kc p) n -> p kc n", p=128)
                for kc in range(KC):
                    nc.sync.dma_start(out=w_sb[:, kc, :], in_=w_re[:, kc, :])
                for mt in range(NVT):
                    ps = ph1_psum(pscnt); pscnt += 1
                    for kc in range(KC):
                        nc.tensor.matmul(
                            ps, xT_sb[:, kc, mt * 128:(mt + 1) * 128],
                            w_sb[:, kc, :],
                            start=(kc == 0), stop=(kc == KC - 1))
                    vt = v_sb[:, mt, :].rearrange("p (h e) -> p h e", e=65)
                    nc.vector.tensor_copy(
                        vt[:, :, 0:64], ps.rearrange("p (h d) -> p h d", d=64))
                    nc.vector.tensor_copy(
                        vt[:, :, 64:65], onecol_f.broadcast_to([128, HLOC, 1]))

            # ---------------- phases 2+3: attention + out-projection ----------------
            with tc.tile_pool(name="attnp", bufs=1) as attnp, \
                 tc.tile_pool(name="probsp", bufs=3) as probsp, \
                 tc.tile_pool(name="drp", bufs=2) as drp, \
                 tc.tile_pool(name="bsp", bufs=2) as bsp, \
                 tc.tile_pool(name="wop", bufs=1) as wop, \
                 tc.tile_pool(name="yp", bufs=3) as yp:
                attn_sb = attnp.tile([128, NMT, T], F32R)   # attn^T: [c_in, t]
                wo_sb = wop.tile([128, NMT, C], F32R)
                wo_re = wo.rearrange("(kc p) n -> p kc n", p=128)
                for kc in range(NMT):
                    nc.sync.dma_start(out=wo_sb[:, kc, :], in_=wo_re[:, kc, :])

                mask3 = mask_f.unsqueeze(1).broadcast_to([128, 2, 128])

                for qt in range(NQT):
                    for mt in range(HLOC // 2):   # head pair (2mt, 2mt+1)
                        nkb = qt * 4 + 4
                        oa = [psp.tile([65, 512], F32, tag="oa",
                                       name=f"oa{qt}_{mt}_{s}") for s in range(2)]
                        for kb in range(nkb):
                            kbl = kb - qt * 4
                            c0 = max(kbl, 0) * 128
                            sc = psp.tile([128, 2, 512], F32, tag="sc")
                            for s in range(2):
                                po = s * 64
                                nc.tensor.matmul(
                                    sc[:, s, c0:512],
                                    kT_sb[po:po + 64, mt, kb * 128:(kb + 1) * 128],
                                    qT_sb[po:po + 64, mt, qt * 512 + c0:(qt + 1) * 512],
                                    start=True, stop=True)
                            pr = probsp.tile([128, 2, 512], F32R, tag="pr")
                            nc.scalar.activation(pr[:, :, c0:512], sc[:, :, c0:512],
                                                 AF.Exp, scale=SCALE)
                            if kbl >= 0:
                                nc.vector.tensor_mul(
                                    pr[:, :, c0:c0 + 128], pr[:, :, c0:c0 + 128], mask3)
                            for s in range(2):
                                h = 2 * mt + s
                                nc.tensor.matmul(
                                    oa[s][:, c0:512],
                                    v_sb[:, kb, h * 65:(h + 1) * 65],
                                    pr[:, s, c0:512],
                                    start=(kb == 0), stop=(kb == nkb - 1))
                        # normalize: attn^T = out^T * (1/denominator)
                        for s in range(2):
                            po = s * 64
                            dr = drp.tile([1, 512], F32R, tag="dr")
                            with nc.allow_low_precision(reason="f32r softmax denom"):
                                nc.vector.reciprocal(dr, oa[s][64:65, :])
                            bc = psp.tile([64, 512], F32, tag="aux")
                            nc.tensor.matmul(bc, ones_r, dr, start=True, stop=True)
                            bs = bsp.tile([64, 512], F32, tag="bs")
                            nc.vector.tensor_copy(bs, bc)
                            nc.vector.tensor_mul(
                                attn_sb[po:po + 64, mt, qt * 512:(qt + 1) * 512],
                                oa[s][0:64, :], bs)
                    # out-projection for the 4 row-tiles of this qt
                    for mtl in range(4):
                        mt3 = qt * 4 + mtl
                        yt = yp.tile([128, C], F32, tag="y")
                        for nt in range(2):
                            ps = psp.tile([128, 512], F32, tag="aux")
                            for kc in range(NMT):
                                nc.tensor.matmul(
                                    ps, attn_sb[:, kc, mt3 * 128:(mt3 + 1) * 128],
                                    wo_sb[:, kc, nt * 512:(nt + 1) * 512],
                                    start=(kc == 0), stop=(kc == NMT - 1))
                            nc.scalar.copy(yt[:, nt * 512:(nt + 1) * 512], ps)
                        nc.sync.dma_start(out=y.ap()[mt3 * 128:(mt3 + 1) * 128, :],
                                          in_=yt)
    nc.compile()
    return nc


def _shard_inputs(x, w_qkv, w_out):
    in_maps = []
    for c in range(8):
        b, hh = c // 2, c % 2
        cols = slice(hh * HD, (hh + 1) * HD)
        in_maps.append({
            "xT": np.ascontiguousarray(x[b].T),
            "wq": np.ascontiguousarray(w_qkv[:, 0 * C:1 * C][:, cols]),
            "wk": np.ascontiguousarray(w_qkv[:, 1 * C:2 * C][:, cols]),
            "wv": np.ascontiguousarray(w_qkv[:, 2 * C:3 * C][:, cols]),
            "wo": np.ascontiguousarray(w_out[hh * HD:(hh + 1) * HD, :]),
        })
    return in_maps


def kernel(x, w_qkv, w_out):
    from concourse.bass_utils import run_bass_kernel_spmd

    x = np.asarray(x, dtype=np.float32)
    w_qkv = np.asarray(w_qkv, dtype=np.float32)
    w_out = np.asarray(w_out, dtype=np.float32)

    if "nc" not in _CACHE:
        _CACHE["nc"] = _build_nc()
    nc = _CACHE["nc"]

    in_maps = _shard_inputs(x, w_qkv, w_out)
    res = run_bass_kernel_spmd(nc, in_maps, core_ids=list(range(8)))
    outs = [res.results[c]["y"] for c in range(8)]
    out = np.stack([outs[2 * b] + outs[2 * b + 1] for b in range(4)])
    return out.astype(np.float32)
